# revision 13
# baseline (speedup 1.0000x reference)
"""3-layer GAT (DGL GATConv semantics) on 8 Trainium2 NeuronCores.

Strategy (graph-parallel, per sharding hint):
  - Host load-balances dst nodes into 8*49 windows of 128 dsts (LPT packing)
    so every core/window has near-equal edge counts; node order is permuted
    accordingly and the output inverse-permuted at the end.
  - Per layer: each core projects its own nodes ([feat|el|er] in one matmul,
    since el = (h@W)@al_diag = h@(W@al_diag)), packs [feat_bf16|el_f32] into a
    768B G-row, AllGathers G across cores.
  - Edge phase per 128-dst window: dma_gather the edges' source G-rows
    (edges pre-sorted by dst into windows; int16 gather indices handled by a
    lo/hi source-offset split), expand er to edges with a tiny PE matmul
    against a host-described staircase one-hot, compute ex=exp(LeakyReLU(el+er))
    batched, then one PE matmul per 128-edge tile accumulates BOTH the
    unnormalized aggregation (ex-weighted feats) and the softmax denominator
    into PSUM (normalization divides at the end; segment-max is unnecessary
    because logits are small and exp cannot overflow fp32).
"""

import math
from contextlib import ExitStack

import numpy as np

import concourse.bass as bass
import concourse.bacc as bacc
import concourse.mybir as mybir
import concourse.tile as tile
from concourse import bass_utils

F32 = mybir.dt.float32
BF16 = mybir.dt.bfloat16
I32 = mybir.dt.int32
I16 = mybir.dt.int16

GROW = 384  # bf16 elements per G row: [feat 256 | el-as-f32 8 | pad] = 768B


class Cfg:
    def __init__(self, n, e, fin, h, dh, ncores, wpc, lo_rows=32768):
        self.N, self.E, self.FIN, self.H, self.DH = n, e, fin, h, dh
        self.HID = h * dh
        self.NCORES, self.WPC = ncores, wpc
        self.NPC = wpc * 128            # padded nodes per core
        self.NPAD = ncores * self.NPC   # padded global nodes
        assert self.NPAD >= n
        self.LO_ROWS = min(lo_rows, self.NPAD)
        self.HI_OFF = max(self.NPAD - self.LO_ROWS, 0)
        self.KT = self.HID // 128       # K tiles for layers 1,2
        self.KT0 = fin // 128           # K tiles for layer 0


def _lpt_windows(deg, cfg):
    """Assign nodes to ncores*wpc windows of exactly 128 slots, balancing
    per-window edge counts (LPT greedy). Returns perm_pos[node] -> global slot."""
    import heapq

    nw = cfg.NCORES * cfg.WPC
    order = np.argsort(-deg, kind="stable")
    heap = [(0, w) for w in range(nw)]
    heapq.heapify(heap)
    counts = np.zeros(nw, np.int64)
    wsum = np.zeros(nw, np.int64)
    assign = np.empty(cfg.N, np.int64)
    stash = []
    for n in order:
        while True:
            s, w = heapq.heappop(heap)
            if counts[w] < 128:
                break
            stash.append(None)  # full window: drop permanently
        assign[n] = w
        counts[w] += 1
        wsum[w] += deg[n]
        if counts[w] < 128:
            heapq.heappush(heap, (wsum[w], w))
    # row within window by assignment order
    perm_pos = np.empty(cfg.N, np.int64)
    next_row = np.zeros(nw, np.int64)
    for n in range(cfg.N):
        w = assign[n]
        r = next_row[w]
        next_row[w] += 1
        perm_pos[n] = w * 128 + r
    return perm_pos


def preprocess(x, edge_index, cfg):
    src = np.asarray(edge_index[0], dtype=np.int64)
    dst = np.asarray(edge_index[1], dtype=np.int64)
    deg = np.bincount(dst, minlength=cfg.N)
    perm_pos = _lpt_windows(deg, cfg)

    psrc = perm_pos[src]
    pdst = perm_pos[dst]
    gw = pdst // 128          # global window of each edge
    row = pdst % 128          # dst row within window

    ncores, wpc = cfg.NCORES, cfg.WPC
    core = gw // wpc
    wi = gw % wpc

    is_lo = psrc < cfg.LO_ROWS

    # Per (core, window): lo/hi edge lists sorted by dst row.
    ek = (core * wpc + wi) * 2 + (~is_lo).astype(np.int64)  # sort key group
    sort_idx = np.lexsort((row, ek))
    s_psrc = psrc[sort_idx]
    s_row = row[sort_idx]
    s_ek = ek[sort_idx]
    grp_start = np.searchsorted(s_ek, np.arange(ncores * wpc * 2 + 1))

    # Uniform-across-cores tile counts per window.
    lo_t = np.zeros(wpc, np.int64)
    hi_t = np.zeros(wpc, np.int64)
    for w in range(wpc):
        for c in range(ncores):
            g = (c * wpc + w) * 2
            nlo = grp_start[g + 1] - grp_start[g]
            nhi = grp_start[g + 2] - grp_start[g + 1]
            lo_t[w] = max(lo_t[w], (nlo + 127) // 128)
            hi_t[w] = max(hi_t[w], (nhi + 127) // 128)
    T = lo_t + hi_t
    TT = int(T.sum())
    toff = np.concatenate([[0], np.cumsum(T)]).astype(np.int64)

    nidx = TT * 8  # int16 idx columns (128 idx per tile / 16 rows)

    idx16 = np.zeros((ncores, 16, nidx), np.int16)
    dstrow = np.full((ncores, 128, TT), -1, np.float32)
    sten = np.zeros((ncores, 128, TT, 2), np.float32)

    for c in range(ncores):
        for w in range(wpc):
            base_t = toff[w]
            for reg in (0, 1):  # 0=lo, 1=hi
                g = (c * wpc + w) * 2 + reg
                lo, hi_ = grp_start[g], grp_start[g + 1]
                e_ps = s_psrc[lo:hi_]
                e_r = s_row[lo:hi_]
                nt = lo_t[w] if reg == 0 else hi_t[w]
                if nt == 0:
                    assert len(e_ps) == 0
                    continue
                cap = nt * 128
                n_e = len(e_ps)
                assert n_e <= cap
                vals = np.zeros(cap, np.int64)
                vals[:n_e] = e_ps if reg == 0 else e_ps - cfg.HI_OFF
                assert (vals >= 0).all() and (vals < cfg.LO_ROWS).all()
                rt0 = base_t + (0 if reg == 0 else lo_t[w])
                # wrapped int16 idx: slot j -> [j%16, col0 + j//16]
                col0 = rt0 * 8
                j = np.arange(cap)
                idx16[c, j % 16, col0 + j // 16] = vals.astype(np.int16)
                # dstrow per slot (pad: -1)
                rows = np.full(cap, -1, np.int64)
                rows[:n_e] = e_r
                dstrow[c, :, rt0:rt0 + nt] = rows.reshape(nt, 128).T
                # staircase bounds per (tile, dst row)
                seg = np.searchsorted(e_r, np.arange(129))  # [129]
                for tl in range(nt):
                    b = tl * 128
                    st = np.clip(seg[:-1] - b, 0, 128)
                    en = np.clip(seg[1:] - b, 0, 128)
                    sten[c, :, rt0 + tl, 0] = st
                    sten[c, :, rt0 + tl, 1] = en

    # per-core permuted x, transposed: [FIN, NPC]
    xT = np.zeros((ncores, cfg.FIN, cfg.NPC), np.float32)
    inv_rows = np.full(cfg.NPAD, -1, np.int64)
    inv_rows[perm_pos] = np.arange(cfg.N)
    xf = np.asarray(x, np.float32)
    for c in range(ncores):
        sl = inv_rows[c * cfg.NPC:(c + 1) * cfg.NPC]
        valid = sl >= 0
        xc = np.zeros((cfg.NPC, cfg.FIN), np.float32)
        xc[valid] = xf[sl[valid]]
        xT[c] = xc.T

    # HW Q7 cores each read their own 16-partition group: replicate.
    idx16 = np.tile(idx16, (1, 8, 1))

    return dict(perm_pos=perm_pos, lo_t=lo_t, hi_t=hi_t, T=T, TT=TT,
                toff=toff, nidx=nidx, idx16=idx16, dstrow=dstrow,
                sten=sten, xT=xT)


def pack_weights(cfg, Ws, als, ars, bs, resW0):
    """[W | W@al_diag | W@ar_diag] K-tiles stacked: [128, 5*, 264]."""
    kts = []
    for l, W in enumerate(Ws):
        ALf = np.zeros((cfg.HID, cfg.H), np.float32)
        ARf = np.zeros((cfg.HID, cfg.H), np.float32)
        for h in range(cfg.H):
            ALf[h * cfg.DH:(h + 1) * cfg.DH, h] = als[l][h]
            ARf[h * cfg.DH:(h + 1) * cfg.DH, h] = ars[l][h]
        Wc = np.concatenate([W, W @ ALf, W @ ARf], axis=1)  # [fin, 264]
        fin = W.shape[0]
        for k in range(fin // 128):
            kts.append(Wc[k * 128:(k + 1) * 128])
    w_all = np.stack(kts)                      # [KT0+2*KT, 128, 264]
    w_all = np.transpose(w_all, (1, 0, 2)).copy()  # [128, nk, 264]
    b_rep = np.stack([np.tile(b[None, :], (128, 1)) for b in bs], axis=1)
    return w_all.astype(np.float32), b_rep.astype(np.float32), \
        resW0.astype(np.float32)


def build_program(cfg, meta, num_cores):
    nc = bacc.Bacc("TRN2", target_bir_lowering=False, debug=False,
                   num_devices=num_cores)
    NPC, HID = cfg.NPC, cfg.HID
    WPC = cfg.WPC
    NK = cfg.KT0 + 2 * cfg.KT
    lo_t, hi_t, T, toff, TT = meta["lo_t"], meta["hi_t"], meta["T"], \
        meta["toff"], meta["TT"]

    d_xT = nc.dram_tensor("xT", [cfg.FIN, NPC], F32, kind="ExternalInput")
    d_w = nc.dram_tensor("w_all", [128, NK, 264], F32, kind="ExternalInput")
    d_rw = nc.dram_tensor("resW0", [128, 256], F32, kind="ExternalInput")
    d_b = nc.dram_tensor("b_rep", [128, 3, 256], F32, kind="ExternalInput")
    d_id = nc.dram_tensor("ident", [128, 128], F32, kind="ExternalInput")
    d_idx = nc.dram_tensor("idx16", [128, meta["nidx"]], I16,
                           kind="ExternalInput")
    d_dr = nc.dram_tensor("dstrow", [128, TT], F32, kind="ExternalInput")
    d_st = nc.dram_tensor("sten", [128, TT, 2], F32, kind="ExternalInput")
    d_out = nc.dram_tensor("out", [NPC, cfg.DH], F32, kind="ExternalOutput")
    debug = getattr(cfg, "debug", False)
    if debug:
        dT0 = int(meta["T"][0])
        d_dbg_fg = nc.dram_tensor("dbg_fg", [128, dT0, GROW], BF16,
                                  kind="ExternalOutput")
        d_dbg_er = nc.dram_tensor("dbg_er", [128, dT0, 4], F32,
                                  kind="ExternalOutput")
        d_dbg_ex = nc.dram_tensor("dbg_ex", [128, dT0, 4], F32,
                                  kind="ExternalOutput")
        d_dbg_pm = nc.dram_tensor("dbg_pm", [128, 260], F32,
                                  kind="ExternalOutput")

    maxT = int(T.max())

    with ExitStack() as ctx:
        tc = ctx.enter_context(tile.TileContext(nc))
        cpool = ctx.enter_context(tc.tile_pool(name="const", bufs=1))
        dram = ctx.enter_context(tc.tile_pool(name="dram", bufs=1,
                                              space="DRAM"))
        fgpool = ctx.enter_context(tc.tile_pool(name="fg", bufs=2))
        ohpool = ctx.enter_context(tc.tile_pool(name="oh", bufs=4))
        mtpool = ctx.enter_context(tc.tile_pool(name="mt", bufs=maxT + 2))
        epool = ctx.enter_context(tc.tile_pool(name="e", bufs=3))
        wpool = ctx.enter_context(tc.tile_pool(name="wt", bufs=3))
        hpool = ctx.enter_context(tc.tile_pool(name="h", bufs=4))
        gpool = ctx.enter_context(tc.tile_pool(name="g", bufs=3))
        ps_m = ctx.enter_context(tc.tile_pool(name="psm", bufs=2,
                                              space="PSUM"))
        ps_e = ctx.enter_context(tc.tile_pool(name="pse", bufs=2,
                                              space="PSUM"))
        ps_p = ctx.enter_context(tc.tile_pool(name="psp", bufs=2,
                                              space="PSUM"))

        g_loc = dram.tile([NPC, GROW], BF16)
        g_fulls = [
            dram.tile([cfg.NPAD, GROW], BF16, name=f"g_full{i}",
                      addr_space="Shared" if num_cores > 4 else "Local")
            for i in range(3)]
        hbuf = [dram.tile([NPC, HID], F32, name="hbuf0"),
                dram.tile([NPC, HID], F32, name="hbuf1")]
        res0 = dram.tile([NPC, HID], F32)

        # resident constants
        w_sb = cpool.tile([128, NK, 264], F32)
        nc.sync.dma_start(w_sb[:], d_w[:])
        rw_sb = cpool.tile([128, 256], F32)
        nc.sync.dma_start(rw_sb[:], d_rw[:])
        b_sb = cpool.tile([128, 3, 256], F32)
        nc.sync.dma_start(b_sb[:], d_b[:])
        id_sb = cpool.tile([128, 128], F32)
        nc.sync.dma_start(id_sb[:], d_id[:])
        idx_sb = cpool.tile([128, meta["nidx"]], I16)
        nc.sync.dma_start(idx_sb[:], d_idx[:])
        dr_sb = cpool.tile([128, TT], F32)
        nc.sync.dma_start(dr_sb[:], d_dr[:])
        st_sb = cpool.tile([128, TT, 2], F32)
        nc.sync.dma_start(st_sb[:], d_st[:])
        er_sb = cpool.tile([128, WPC, 4], F32)
        iota_sb = cpool.tile([128, 128], F32)
        nc.gpsimd.iota(iota_sb[:], pattern=[[1, 128]], base=0,
                       channel_multiplier=0,
                       allow_small_or_imprecise_dtypes=True)

        kt_of_layer = [list(range(cfg.KT0)),
                       list(range(cfg.KT0, cfg.KT0 + cfg.KT)),
                       list(range(cfg.KT0 + cfg.KT, NK))]

        for l in range(3):
            # ---------------- projection phase ----------------
            for nt in range(WPC):
                kts = kt_of_layer[l]
                lhsTs = []
                if l == 0:
                    xt = hpool.tile([128, 128], F32, tag="lhsT")
                    nc.sync.dma_start(xt[:], d_xT[:, nt * 128:(nt + 1) * 128])
                    lhsTs.append(xt)
                else:
                    h_in = hpool.tile([128, HID], F32, tag="hin")
                    nc.sync.dma_start(
                        h_in[:], hbuf[(l + 1) % 2][nt * 128:(nt + 1) * 128, :])
                    for ft in range(cfg.KT):
                        pst = ps_p.tile([128, 128], F32, tag="pt")
                        nc.tensor.transpose(
                            pst[:], h_in[:, ft * 128:(ft + 1) * 128], id_sb[:])
                        hT = hpool.tile([128, 128], F32, tag="lhsT")
                        nc.vector.tensor_copy(hT[:], pst[:])
                        lhsTs.append(hT)
                pp = ps_p.tile([128, 264], F32, tag="pp")
                for k, (kt, lt) in enumerate(zip(kts, lhsTs)):
                    nc.tensor.matmul(pp[:], lt[:], w_sb[:, kt, :],
                                     start=(k == 0), stop=(k == len(kts) - 1))
                g_sb = gpool.tile([128, GROW], BF16)
                nc.vector.tensor_copy(g_sb[:, 0:256], pp[:, 0:256])
                nc.vector.tensor_copy(g_sb[:, 256:264].bitcast(F32),
                                      pp[:, 256:260])
                nc.vector.memset(g_sb[:, 264:GROW], 0)
                nc.vector.tensor_copy(er_sb[:, nt, :], pp[:, 260:264])
                nc.sync.dma_start(g_loc[nt * 128:(nt + 1) * 128, :], g_sb[:])
                if l == 0:
                    pr = ps_p.tile([128, 256], F32, tag="pp")
                    nc.tensor.matmul(pr[:], lhsTs[0][:], rw_sb[:],
                                     start=True, stop=True)
                    r_sb = gpool.tile([128, 256], F32, tag="res")
                    nc.vector.tensor_copy(r_sb[:], pr[:])
                    nc.sync.dma_start(res0[nt * 128:(nt + 1) * 128, :],
                                      r_sb[:])

            g_full = g_fulls[l]
            nc.gpsimd.collective_compute(
                "AllGather", mybir.AluOpType.bypass,
                replica_groups=[list(range(num_cores))],
                ins=[g_loc.opt()], outs=[g_full.opt()])

            # ---------------- aggregation phase ----------------
            for w in range(WPC):
                Tw = int(T[w])
                lt_, ht_ = int(lo_t[w]), int(hi_t[w])
                t0 = int(toff[w])
                fg = fgpool.tile([128, maxT, GROW], BF16)
                if lt_ > 0:
                    nc.gpsimd.dma_gather(
                        out_ap=fg[:, 0:lt_, :],
                        in_ap=g_full[0:cfg.LO_ROWS, :],
                        idxs_ap=idx_sb[:, t0 * 8:(t0 + lt_) * 8],
                        num_idxs=lt_ * 128, num_idxs_reg=lt_ * 128,
                        elem_size=GROW, single_packet=(lt_ * 128 <= 1024))
                if ht_ > 0:
                    nc.gpsimd.dma_gather(
                        out_ap=fg[:, lt_:Tw, :],
                        in_ap=g_full[cfg.HI_OFF:cfg.NPAD, :],
                        idxs_ap=idx_sb[:, (t0 + lt_) * 8:(t0 + Tw) * 8],
                        num_idxs=ht_ * 128, num_idxs_reg=ht_ * 128,
                        elem_size=GROW, single_packet=(ht_ * 128 <= 1024))

                pe = ps_e.tile([128, maxT, 4], F32)
                mts = []
                for t in range(Tw):
                    wt = t0 + t
                    mst = ohpool.tile([128, 128], F32, tag="mst")
                    nc.vector.tensor_scalar(
                        mst[:], iota_sb[:], st_sb[:, wt, 0:1], None,
                        mybir.AluOpType.is_ge)
                    nc.vector.scalar_tensor_tensor(
                        mst[:], iota_sb[:], st_sb[:, wt, 1:2], mst[:],
                        mybir.AluOpType.is_lt, mybir.AluOpType.mult)
                    nc.tensor.matmul(pe[:, t, :], mst[:], er_sb[:, w, :],
                                     start=True, stop=True)
                    mt = mtpool.tile([128, 128], BF16, tag="mt")
                    nc.vector.tensor_tensor(
                        mt[:], dr_sb[:, wt:wt + 1].broadcast_to([128, 128]),
                        iota_sb[:], mybir.AluOpType.is_equal)
                    mts.append(mt)

                el_v = fg[:, 0:Tw, 256:264].bitcast(F32)    # [128, Tw, 4]
                e_sb = epool.tile([128, maxT, 4], F32, tag="e")
                nc.vector.tensor_tensor(e_sb[:, 0:Tw, :], el_v,
                                        pe[:, 0:Tw, :], mybir.AluOpType.add)
                nc.vector.scalar_tensor_tensor(
                    e_sb[:, 0:Tw, :], e_sb[:, 0:Tw, :], 0.2, e_sb[:, 0:Tw, :],
                    mybir.AluOpType.mult, mybir.AluOpType.max)
                ex_sb = epool.tile([128, maxT, 4], F32, tag="ex")
                nc.scalar.activation(ex_sb[:, 0:Tw, :], e_sb[:, 0:Tw, :],
                                     mybir.ActivationFunctionType.Exp)
                exb = epool.tile([128, maxT, 4], BF16, tag="exb")
                nc.vector.tensor_copy(exb[:, 0:Tw, :], ex_sb[:, 0:Tw, :])

                pm = ps_m.tile([128, 260], F32)
                for t in range(Tw):
                    wsb = wpool.tile([128, 256], BF16)
                    nc.vector.tensor_tensor(
                        wsb[:].rearrange("p (h d) -> p h d", h=4),
                        fg[:, t, 0:256].rearrange("p (h d) -> p h d", h=4),
                        exb[:, t, :].unsqueeze(2).broadcast_to([128, 4, 64]),
                        mybir.AluOpType.mult)
                    nc.tensor.matmul(pm[:, 0:256], mts[t][:], wsb[:],
                                     start=(t == 0), stop=(t == Tw - 1),
                                     skip_group_check=True)
                for t in range(Tw):
                    nc.tensor.matmul(pm[:, 256:260], mts[t][:], exb[:, t, :],
                                     start=(t == 0), stop=(t == Tw - 1),
                                     skip_group_check=True)

                if debug and l == 0 and w == 0:
                    nc.sync.dma_start(d_dbg_fg[:], fg[:, 0:Tw, :])
                    nc.sync.dma_start(d_dbg_ex[:], ex_sb[:, 0:Tw, :])
                    er_dbg = epool.tile([128, maxT, 4], F32, tag="erdbg")
                    nc.vector.tensor_copy(er_dbg[:, 0:Tw, :], pe[:, 0:Tw, :])
                    nc.sync.dma_start(d_dbg_er[:], er_dbg[:, 0:Tw, :])
                    pm_dbg = epool.tile([128, 260], F32, tag="pmdbg")
                    nc.vector.tensor_copy(pm_dbg[:], pm[:])
                    nc.sync.dma_start(d_dbg_pm[:], pm_dbg[:])
                den = epool.tile([128, 4], F32, tag="den")
                nc.vector.tensor_scalar(den[:], pm[:, 256:260], 1e-16, None,
                                        mybir.AluOpType.max)
                rden = epool.tile([128, 4], F32, tag="rden")
                nc.vector.reciprocal(rden[:], den[:])
                hn = hpool.tile([128, HID], F32, tag="hn")
                nc.vector.tensor_tensor(
                    hn[:].rearrange("p (h d) -> p h d", h=4),
                    pm[:, 0:256].rearrange("p (h d) -> p h d", h=4),
                    rden[:].unsqueeze(2).broadcast_to([128, 4, 64]),
                    mybir.AluOpType.mult)
                rsb = hpool.tile([128, HID], F32, tag="res_in")
                rsrc = res0 if l == 0 else hbuf[(l + 1) % 2]
                nc.sync.dma_start(rsb[:], rsrc[w * 128:(w + 1) * 128, :])
                nc.vector.tensor_tensor(hn[:], hn[:], rsb[:],
                                        mybir.AluOpType.add)
                nc.vector.tensor_tensor(hn[:], hn[:], b_sb[:, l, :],
                                        mybir.AluOpType.add)
                if l < 2:
                    nc.scalar.activation(hn[:], hn[:],
                                         mybir.ActivationFunctionType.Relu)
                    nc.sync.dma_start(hbuf[l % 2][w * 128:(w + 1) * 128, :],
                                      hn[:])
                else:
                    osb = hpool.tile([128, cfg.DH], F32, tag="osb")
                    nc.vector.tensor_reduce(
                        osb[:],
                        hn[:].rearrange("p (h d) -> p d h", h=4),
                        mybir.AxisListType.X, mybir.AluOpType.add)
                    nc.vector.tensor_scalar(osb[:], osb[:], 1.0 / cfg.H, None,
                                            mybir.AluOpType.mult)
                    nc.sync.dma_start(d_out[w * 128:(w + 1) * 128, :], osb[:])

    nc.compile()
    return nc


def make_in_maps(cfg, meta, wnp, num_cores):
    w_all, b_rep, rw = wnp
    ident = np.eye(128, dtype=np.float32)
    maps = []
    for c in range(num_cores):
        maps.append({
            "xT": meta["xT"][c],
            "w_all": w_all, "resW0": rw, "b_rep": b_rep, "ident": ident,
            "idx16": meta["idx16"][c],
            "dstrow": meta["dstrow"][c],
            "sten": meta["sten"][c].reshape(128, meta["TT"], 2),
        })
    return maps


def assemble_output(cfg, meta, results):
    out = np.empty((cfg.N, cfg.DH), np.float32)
    pp = meta["perm_pos"]
    full = np.concatenate([r["out"] for r in results], axis=0)  # [NPAD, DH]
    out[:] = full[pp]
    return out


def solve(x, edge_index, Ws, als, ars, bs, resW0, cfg, trace=False):
    meta = preprocess(x, edge_index, cfg)
    wnp = pack_weights(cfg, Ws, als, ars, bs, resW0)
    nc = build_program(cfg, meta, cfg.NCORES)
    in_maps = make_in_maps(cfg, meta, wnp, cfg.NCORES)
    res = bass_utils.run_bass_kernel_spmd(
        nc, in_maps, core_ids=list(range(cfg.NCORES)), trace=trace)
    out = assemble_output(cfg, meta, res.results)
    return out, res


def kernel(x, edge_index, W0, W1, W2, al0, al1, al2, ar0, ar1, ar2,
           b0, b1, b2, resW0):
    cfg = Cfg(n=50000, e=800000, fin=128, h=4, dh=64, ncores=8, wpc=49)
    out, _ = solve(np.asarray(x, np.float32), np.asarray(edge_index),
                   [np.asarray(W0, np.float32), np.asarray(W1, np.float32),
                    np.asarray(W2, np.float32)],
                   [np.asarray(al0, np.float32), np.asarray(al1, np.float32),
                    np.asarray(al2, np.float32)],
                   [np.asarray(ar0, np.float32), np.asarray(ar1, np.float32),
                    np.asarray(ar2, np.float32)],
                   [np.asarray(b0, np.float32), np.asarray(b1, np.float32),
                    np.asarray(b2, np.float32)],
                   np.asarray(resW0, np.float32), cfg)
    return out
